# revision 12
# baseline (speedup 1.0000x reference)
"""Directional Chamfer distance kernel for Trainium2 (8 NeuronCores).

Computes sum_m min_n ||t_m - s_n||^2 for template points t (M=10000) and
scan points s (N=20000), all in 3D.

Strategy
--------
- Shard template points (rows of the MxN distance matrix) across the 8
  cores: 1250 rows each (padded to 1280 = 10 blocks of 128). The scan
  cloud is replicated to every core; each core's partial row-minima are
  summed on the host (the trivial "all-reduce" of this sharding).
- d2[m,n] = t_sq[m] + s_sq[n] - 2 t.s is linear in an augmented K=5
  contraction (zero-padded to K=32): lhsT rows = [-2tx, -2ty, -2tz, 1,
  t_sq, 0...], rhs rows = [sx, sy, sz, s_sq, 1, 0...]. One matmul per
  (128-row m-block, 512-col n-chunk) streams raw squared distances into
  PSUM.
- The 4 matmuls of a "quad group" (4 n-chunks) go to 4 distinct 32-row
  groups of the PE array (tile_position) so they run concurrently —
  fp32 matmuls are ~4x slower than bf16 serially (1943ns vs 492ns per
  512-col group, measured), and row-tiling hides that entirely.
- PSUM drain is the bottleneck: only DVE+ACT can read PSUM, 1 elem per
  lane-cycle each. Per quad group, ACT copies two banks to SBUF
  (~302ns) while DVE consumes the other two banks paired with that SBUF
  copy via one fused tensor_tensor_scan (state = min(psum[t], state,
  sbuf[t]); ~1099ns for [128,1024], i.e. 2 fresh elements per
  lane-cycle). Scans chain across groups via initial=prev_out[:, -1:].
- Row minima are clamped at 0 (matches the reference's elementwise
  clamp; max(.,0) commutes with min) and DMA'd out per m-block column.
"""

from contextlib import ExitStack

import numpy as np

import concourse.bacc as bacc
import concourse.tile as tile
from concourse import mybir
from concourse.bass_utils import run_bass_kernel_spmd

N_CORES = 8
NCHUNK = 512          # matmul free dim = one PSUM bank of fp32
KROWS = 32            # padded contraction rows per PE row-group
KAUG = 5              # used rows: -2x,-2y,-2z, 1, t_sq


def _build_program(m_pad: int, n_pad: int, repeat: int = 1):
    """Build the Bass/Tile program for one core: [m_pad] template rows
    (multiple of 128) against [n_pad] scan points (multiple of 2048).
    repeat>1 wraps the whole compute in a For_i loop (for benchmarking)."""
    m_blocks = m_pad // 128
    n_groups = n_pad // (4 * NCHUNK)   # quad groups per m-block
    slot_w = n_groups * NCHUNK         # free width of rhs per row-group

    nc = bacc.Bacc("TRN2")
    # combined per-row-group input: cols [0, m_pad) = lhsT (weights),
    # cols [m_pad, m_pad+slot_w) = rhs. One DMA per row-group half so a
    # PE instruction never needs more than one DMA semaphore wait.
    inp_h = nc.dram_tensor("inp", [4 * KROWS, m_pad + slot_w],
                           mybir.dt.float32, kind="ExternalInput")
    out_h = nc.dram_tensor("out", [128, m_blocks], mybir.dt.float32,
                           kind="ExternalOutput")

    with tile.TileContext(nc) as tc:
        with ExitStack() as ctx:
            _emit(ctx, tc, nc, inp_h, out_h, m_pad, m_blocks, n_groups,
                  slot_w, repeat)
    nc.compile()
    return nc


def _emit(ctx, tc, nc, inp_h, out_h, m_pad, m_blocks, n_groups, slot_w,
          repeat):
    fp32 = mybir.dt.float32
    Alu = mybir.AluOpType

    consts = ctx.enter_context(tc.tile_pool(name="consts", bufs=1))
    pa = ctx.enter_context(tc.tile_pool(name="pa", bufs=2, space="PSUM"))
    pb = ctx.enter_context(tc.tile_pool(name="pb", bufs=2, space="PSUM"))
    s_pool = ctx.enter_context(tc.tile_pool(name="spool", bufs=3))
    scr_pool = ctx.enter_context(tc.tile_pool(name="scr", bufs=3))

    # SBUF-resident combined input; row-group j's rows live at partitions
    # 32j..32j+31 (rows 5..31 are zeros). Split DMAs for load/compute overlap.
    W = m_pad + slot_w
    comb = consts.tile([128, W], fp32)
    cut = m_pad + (slot_w // 2)
    for j in range(4):
        nc.sync.dma_start(
            out=comb[32 * j:32 * (j + 1), 0:cut],
            in_=inp_h[KROWS * j:KROWS * (j + 1), 0:cut],
        )
        nc.sync.dma_start(
            out=comb[32 * j:32 * (j + 1), cut:W],
            in_=inp_h[KROWS * j:KROWS * (j + 1), cut:W],
        )

    nearest = consts.tile([128, m_blocks], fp32)

    def body(_iv=None):
        for i in range(m_blocks):
            prev_scr = None
            for g in range(n_groups):
                ta = pa.tile([128, 1024], fp32)
                tb = pb.tile([128, 1024], fp32)
                for j, (dst, h) in enumerate(
                        ((ta, 0), (ta, 1), (tb, 0), (tb, 1))):
                    nc.tensor.matmul(
                        out=dst[:, 512 * h:512 * (h + 1)],
                        lhsT=comb[32 * j:32 * (j + 1),
                                  128 * i:128 * (i + 1)],
                        rhs=comb[32 * j:32 * (j + 1),
                                 m_pad + NCHUNK * g:
                                 m_pad + NCHUNK * (g + 1)],
                        start=True, stop=True,
                        tile_position=(32 * j, 0),
                    )
                s_tile = s_pool.tile([128, 1024], fp32)
                nc.scalar.copy(out=s_tile[:, :], in_=tb[:, :])
                scr = scr_pool.tile([128, 1024], fp32)
                init = 3.0e38 if g == 0 else prev_scr[:, 1023:1024]
                nc.vector.tensor_tensor_scan(
                    out=scr[:, :], data0=ta[:, :], data1=s_tile[:, :],
                    initial=init, op0=Alu.min, op1=Alu.min)
                prev_scr = scr
            # clamp at 0 (reference clamps d2 elementwise; min/relu commute)
            nc.vector.tensor_scalar_max(
                out=nearest[:, i:i + 1], in0=prev_scr[:, 1023:1024],
                scalar1=0.0)

    if repeat == 1:
        body()
    else:
        tc.For_i_unrolled(0, repeat, 1, body, max_unroll=1)

    nc.sync.dma_start(out=out_h[:, :], in_=nearest[:, :])


def _prep_inputs(scan_vertices, template_vertices, m_pad, n_pad):
    """Host-side shard + augment. Returns per-core input maps."""
    s = np.asarray(scan_vertices, dtype=np.float32)
    t = np.asarray(template_vertices, dtype=np.float32)
    n = s.shape[0]
    m = t.shape[0]
    m_loc = (m + N_CORES - 1) // N_CORES
    m_blocks = m_pad // 128
    n_groups = n_pad // (4 * NCHUNK)
    slot_w = n_groups * NCHUNK

    # augmented scan rows [5, n_pad]: sx, sy, sz, s_sq, 1; pads: huge s_sq
    aug_s = np.zeros((KAUG, n_pad), dtype=np.float32)
    aug_s[0:3, :n] = s.T
    aug_s[3, :n] = (s * s).sum(-1)
    aug_s[3, n:] = 1.0e30
    aug_s[4, :] = 1.0
    # chunk c = 4g+j -> row-group j, cols [512g, 512g+512)
    # rhs[j, k, g, :] = aug_s[k, chunk 4g+j]
    rhs = (aug_s.reshape(KAUG, n_groups, 4, NCHUNK)
           .transpose(2, 0, 1, 3)
           .reshape(4, KAUG, slot_w))

    in_maps = []
    for c in range(N_CORES):
        tc_ = t[c * m_loc:min((c + 1) * m_loc, m)]
        k = tc_.shape[0]
        aug_t = np.zeros((KAUG, m_pad), dtype=np.float32)
        aug_t[0:3, :k] = -2.0 * tc_.T
        aug_t[3, :k] = 1.0
        aug_t[4, :k] = (tc_ * tc_).sum(-1)
        inp = np.zeros((4, KROWS, m_pad + slot_w), dtype=np.float32)
        inp[:, :KAUG, :m_pad] = aug_t[None, :, :]
        inp[:, :KAUG, m_pad:] = rhs
        in_maps.append({"inp": inp.reshape(4 * KROWS, m_pad + slot_w)})
    return in_maps


_CACHE = {}


def _get_program(m_pad, n_pad, repeat=1):
    key = (m_pad, n_pad, repeat)
    if key not in _CACHE:
        _CACHE[key] = _build_program(m_pad, n_pad, repeat)
    return _CACHE[key]


def run(scan_vertices, template_vertices, m_pad=1280, n_pad=20480, **kw):
    """Run the sharded kernel; returns (scalar_sum, BassKernelResults)."""
    in_maps = _prep_inputs(scan_vertices, template_vertices, m_pad, n_pad)
    nc = _get_program(m_pad, n_pad)
    res = run_bass_kernel_spmd(nc, in_maps, core_ids=list(range(N_CORES)),
                               **kw)
    total = 0.0
    for c in range(N_CORES):
        total += float(res.results[c]["out"].sum(dtype=np.float64))
    return np.float32(total), res


def kernel(scan_vertices, template_vertices):
    out, _ = run(scan_vertices, template_vertices)
    return out
